# revision 1
# baseline (speedup 1.0000x reference)
"""LRU (diagonal complex linear recurrence) Trainium2 Bass kernel, v2.

Math (per batch b, channel h, time t = 0..L-1):
    u_t   = delta * (x_t @ B_real + i * x_t @ B_img)
    h_t   = lam * h_{t-1} + u_t,   h_{-1} = h0,  lam = r e^{i theta}
    out_t = Re(h_t)

Polar trick: h_t = e^{i theta (t+1)} g_t with g_t = r g_{t-1} + e^{-i theta(t+1)} u_t,
g_{-1} = h0. r is REAL so Re/Im decouple into two real first-order scans ->
native DVE tensor_tensor_scan (fp32 internal state; data0 r kept fp32 so decay
error does not compound). Rotation tables cos/sin(theta*(t+1)) precomputed
host-side in float64, stored fp16.

v2: fp16 datapath. x is cast to fp16 on host and transposed by the DMA xbar
(dma_start_transpose) straight into SBUF, GEMM runs fp16 (full PE rate, FWL),
rotations run fp16 on DVE (2x mode) split with GPSIMD, output transposed back
by PE (fp16, 1 cyc/row), upcast to fp32 by ScalarE on the PSUM->SBUF copy.

Sharding: batch-parallel over 8 cores (2 batch elements each), SPMD via
run_bass_kernel_spmd.
"""

import os
from contextlib import ExitStack

import numpy as np

import concourse.bass as bass
import concourse.tile as tile
from concourse import bacc, mybir
from concourse.masks import make_identity

B, L, F, H = 16, 4096, 512, 512
N_CORES = 8
B_LOC = B // N_CORES
HG = H // 128
FG = F // 128
TC = 512
NTC = L // TC
FP32 = mybir.dt.float32
F16 = mybir.dt.float16

ABLATE = set(os.environ.get("LRU_ABLATE", "").split(","))
A = mybir.AluOpType


def build_program():
    nc = bacc.Bacc("TRN2", target_bir_lowering=False, debug=False,
                   enable_asserts=False, num_devices=1)

    x_d = nc.dram_tensor("x", [B_LOC, L, F], F16, kind="ExternalInput").ap()
    br_d = nc.dram_tensor("btr", [F, H], F16, kind="ExternalInput").ap()
    bi_d = nc.dram_tensor("bti", [F, H], F16, kind="ExternalInput").ap()
    r_d = nc.dram_tensor("rvec", [H], FP32, kind="ExternalInput").ap()
    cos_d = nc.dram_tensor("ctab", [H, L], F16, kind="ExternalInput").ap()
    sin_d = nc.dram_tensor("stab", [H, L], F16, kind="ExternalInput").ap()
    h0r_d = nc.dram_tensor("h0r", [H], FP32, kind="ExternalInput").ap()
    h0i_d = nc.dram_tensor("h0i", [H], FP32, kind="ExternalInput").ap()
    out_d = nc.dram_tensor("out", [B_LOC, L, H], FP32, kind="ExternalOutput").ap()

    with tile.TileContext(nc) as tc, ExitStack() as ctx:
        singles = ctx.enter_context(tc.tile_pool(name="singles", bufs=1))
        xt_pool = ctx.enter_context(tc.tile_pool(name="xt", bufs=1))
        tab_pool = ctx.enter_context(tc.tile_pool(name="tabs", bufs=3))
        work = ctx.enter_context(tc.tile_pool(name="work", bufs=3))
        ps_mm = ctx.enter_context(tc.tile_pool(name="ps_mm", bufs=2, space="PSUM"))
        ps_out = ctx.enter_context(tc.tile_pool(name="ps_out", bufs=2, space="PSUM"))

        ident = singles.tile([128, 128], F16)
        make_identity(nc, ident)

        btr_s = singles.tile([128, FG, H], F16)
        bti_s = singles.tile([128, FG, H], F16)
        nc.sync.dma_start(out=btr_s, in_=br_d.rearrange("(fg p) h -> p fg h", p=128))
        nc.sync.dma_start(out=bti_s, in_=bi_d.rearrange("(fg p) h -> p fg h", p=128))

        r_s = singles.tile([128, HG], FP32)
        h0r_s = singles.tile([128, HG], FP32)
        h0i_s = singles.tile([128, HG], FP32)
        nc.sync.dma_start(out=r_s, in_=r_d.rearrange("(hg p) -> p hg", p=128))
        nc.sync.dma_start(out=h0r_s, in_=h0r_d.rearrange("(hg p) -> p hg", p=128))
        nc.sync.dma_start(out=h0i_s, in_=h0i_d.rearrange("(hg p) -> p hg", p=128))
        ones = singles.tile([128, TC], FP32)
        nc.vector.memset(ones, 1.0)
        r_bc = singles.tile([128, HG, TC], FP32)
        for hg in range(HG):
            nc.vector.tensor_scalar(r_bc[:, hg, :], ones, r_s[:, hg:hg + 1],
                                    None, op0=A.mult)

        # x transposed into SBUF via DMA xbar: xt[b][fg] = (128 f, L t) fp16
        xt = [[xt_pool.tile([128, L], F16, tag=f"xt{b}_{fg}", name=f"xt{b}_{fg}")
               for fg in range(FG)] for b in range(B_LOC)]
        for b in range(B_LOC):
            for fg in range(FG):
                for tcn in range(NTC):
                    sl = slice(tcn * TC, (tcn + 1) * TC)
                    nc.sync.dma_start_transpose(
                        xt[b][fg][:, sl],
                        x_d[b, sl, fg * 128:(fg + 1) * 128])

        for hg in range(HG):
            hsl = slice(hg * 128, (hg + 1) * 128)
            gprev = {}
            for tcn in range(NTC):
                sl = slice(tcn * TC, (tcn + 1) * TC)
                ct = tab_pool.tile([128, TC], F16, tag="ct")
                st = tab_pool.tile([128, TC], F16, tag="st")
                if "tab" not in ABLATE:
                    nc.sync.dma_start(out=ct, in_=cos_d[hsl, sl])
                    nc.sync.dma_start(out=st, in_=sin_d[hsl, sl])

                for b in range(B_LOC):
                    pur = ps_mm.tile([128, TC], FP32, tag="pur")
                    pui = ps_mm.tile([128, TC], FP32, tag="pui")
                    if "gemm" in ABLATE:
                        nc.vector.memset(pur, 0.0)
                        nc.vector.memset(pui, 0.0)
                    else:
                        for fg in range(FG):
                            nc.tensor.matmul(pur, btr_s[:, fg, hsl],
                                             xt[b][fg][:, sl],
                                             start=(fg == 0), stop=(fg == FG - 1))
                        for fg in range(FG):
                            nc.tensor.matmul(pui, bti_s[:, fg, hsl],
                                             xt[b][fg][:, sl],
                                             start=(fg == 0), stop=(fg == FG - 1))

                    # stage + cast to fp16 on ScalarE (frees DVE, enables 2x)
                    ur = work.tile([128, TC], F16, tag="ur")
                    ui = work.tile([128, TC], F16, tag="ui")
                    nc.scalar.copy(out=ur, in_=pur)
                    nc.scalar.copy(out=ui, in_=pui)

                    # vr = c*ur + s*ui ; vi = c*ui - s*ur  (DVE/POOL split)
                    t1 = work.tile([128, TC], F16, tag="t1")
                    t2 = work.tile([128, TC], F16, tag="t2")
                    t3 = work.tile([128, TC], F16, tag="t3")
                    t4 = work.tile([128, TC], F16, tag="t4")
                    vr = work.tile([128, TC], F16, tag="vr")
                    vi = work.tile([128, TC], F16, tag="vi")
                    if "rot" in ABLATE:
                        nc.vector.tensor_copy(vr, ur)
                        nc.vector.tensor_copy(vi, ui)
                    else:
                        nc.vector.tensor_mul(t1, ct, ur)
                        nc.gpsimd.tensor_tensor(t2, st, ui, op=A.mult)
                        nc.vector.tensor_mul(t3, ct, ui)
                        nc.gpsimd.tensor_tensor(t4, st, ur, op=A.mult)
                        nc.vector.tensor_add(vr, t1, t2)
                        nc.gpsimd.tensor_tensor(vi, t3, t4, op=A.subtract)

                    gr = work.tile([128, TC], F16, tag=f"gr{b}")
                    gi = work.tile([128, TC], F16, tag=f"gi{b}")
                    if tcn == 0:
                        init_r, init_i = h0r_s[:, hg:hg + 1], h0i_s[:, hg:hg + 1]
                    else:
                        gr_p, gi_p = gprev[b]
                        init_r, init_i = gr_p[:, TC - 1:TC], gi_p[:, TC - 1:TC]
                    if "scan" in ABLATE:
                        nc.vector.tensor_copy(gr, vr)
                        nc.vector.tensor_copy(gi, vi)
                    else:
                        nc.vector.tensor_tensor_scan(gr, r_bc[:, hg, :], vr, init_r,
                                                     op0=A.mult, op1=A.add)
                        nc.vector.tensor_tensor_scan(gi, r_bc[:, hg, :], vi, init_i,
                                                     op0=A.mult, op1=A.add)
                    gprev[b] = (gr, gi)

                    # out = c*gr - s*gi
                    o1 = work.tile([128, TC], F16, tag="o1")
                    o2 = work.tile([128, TC], F16, tag="o2")
                    res = work.tile([128, TC], F16, tag="res")
                    if "orot" in ABLATE:
                        nc.vector.tensor_copy(res, gr)
                    else:
                        nc.vector.tensor_mul(o1, ct, gr)
                        nc.gpsimd.tensor_tensor(o2, st, gi, op=A.mult)
                        nc.gpsimd.tensor_tensor(res, o1, o2, op=A.subtract)

                    if "out" not in ABLATE:
                        pres = ps_out.tile([128, TC], F16, tag="pres")
                        for sb in range(TC // 128):
                            nc.tensor.transpose(
                                pres[:, sb * 128:(sb + 1) * 128],
                                res[:, sb * 128:(sb + 1) * 128], ident)
                        rest = work.tile([128, TC], FP32, tag="rest")
                        nc.scalar.copy(out=rest, in_=pres)
                        nc.sync.dma_start(
                            out=out_d[b, sl, hsl].rearrange("(sb p) h -> p sb h",
                                                            p=128),
                            in_=rest)

    nc.compile()
    return nc


def _prepare(inputs):
    x = np.asarray(inputs["x"], dtype=np.float32)
    B_real = np.asarray(inputs["B_real"], dtype=np.float32)
    B_img = np.asarray(inputs["B_img"], dtype=np.float32)
    nu = np.asarray(inputs["nu"], dtype=np.float64)
    theta = np.asarray(inputs["theta"], dtype=np.float64)
    delta = np.asarray(inputs["delta"], dtype=np.float32)
    h0r = np.asarray(inputs["h0_real"], dtype=np.float32)
    h0i = np.asarray(inputs["h0_img"], dtype=np.float32)

    btr = np.ascontiguousarray(B_real * delta[None, :]).astype(np.float16)
    bti = np.ascontiguousarray(B_img * delta[None, :]).astype(np.float16)
    r = np.exp(-np.exp(nu)).astype(np.float32)
    ang = theta[:, None] * np.arange(1, L + 1, dtype=np.float64)[None, :]
    ctab = np.cos(ang).astype(np.float16)
    stab = np.sin(ang).astype(np.float16)
    return (x.astype(np.float16), btr, bti, r, ctab, stab, h0r, h0i)


_NC_CACHE = {}


def get_program():
    if "nc" not in _NC_CACHE:
        _NC_CACHE["nc"] = build_program()
    return _NC_CACHE["nc"]


def make_in_maps(inputs):
    x, btr, bti, r, ctab, stab, h0r, h0i = _prepare(inputs)
    shared = dict(btr=btr, bti=bti, rvec=r, ctab=ctab, stab=stab,
                  h0r=h0r, h0i=h0i)
    return [dict(x=np.ascontiguousarray(x[c * B_LOC:(c + 1) * B_LOC]), **shared)
            for c in range(N_CORES)]


def kernel(**inputs) -> np.ndarray:
    from concourse.bass_utils import run_bass_kernel_spmd

    nc = get_program()
    in_maps = make_in_maps(inputs)
    res = run_bass_kernel_spmd(nc, in_maps, list(range(N_CORES)))
    out = np.empty((B, L, H), dtype=np.float32)
    for c in range(N_CORES):
        out[c * B_LOC:(c + 1) * B_LOC] = res.results[c]["out"]
    return out



# revision 4
# speedup vs baseline: 1.6420x; 1.6420x over previous
"""LRU (diagonal complex linear recurrence) Trainium2 Bass kernel, v4.

Math (per batch b, channel h, time t = 0..L-1):
    u_t   = delta * (x_t @ B_real + i * x_t @ B_img)
    h_t   = lam * h_{t-1} + u_t,   h_{-1} = h0,  lam = r e^{i theta}
    out_t = Re(h_t)

v4 structure — time-decimated (m=2) polar scan with the pair-combine
folded into the GEMM:
  E_j := h_{2j} obeys E_j = lam^2 E_{j-1} + u~_j with
  u~_j = lam*u_{2j-1} + u_{2j} = x_{2j-1} @ (lam*Bd) + x_{2j} @ Bd.
  So a GEMM over pairs (x_{2j-1}, x_{2j}) with weight sets
  {btr2,bti2} = lam*(btr+i*bti) and {btr,bti} yields u~ directly at
  half resolution. Polar trick on lam^2 = r^2 e^{i*2theta}:
  E_j = e^{i*2theta(j+1)} G_j,  G_j = r^2 G_{j-1} + e^{-i*2theta(j+1)} u~_j,
  G_{-1} = h0 / lam  (host-computed, fp32).
  Even outputs: out_{2j}  = C.Gr - S.Gi           (C,S = cos/sin 2theta(j+1))
  Odd  outputs: out_{2j+1} = P1.Gr - P2.Gi + Re(u_{2j+1})
                (P1,P2 = r*cos/sin(theta(2j+3)); Re(u_odd) from a third
                 partial GEMM x_{2j+1} @ btr).
  Scans run on DVE at half resolution (scan is the only serial resource);
  rotations split DVE/GpSimd fp16 2x; PSUM->SBUF staging on ScalarE.
  Output written as [b, h, t] (contiguous 8KB rows), transposed on host.

Sharding: batch-parallel over 8 cores (2 batch elements each), SPMD.
"""

from contextlib import ExitStack

import numpy as np

import concourse.bass as bass
import concourse.tile as tile
from concourse import bacc, mybir

B, L, F, H = 16, 4096, 512, 512
N_CORES = 8
B_LOC = B // N_CORES
HG = H // 128
FG = F // 128
J = L // 2            # half-res length
JO = J + 1            # odd x stream incl. left pad
JOP = 2064            # padded to multiple of 16 for xbar transpose
W = 1024              # elementwise tile width (j-cols)
NW = J // W           # 2 elementwise tiles per (hg, b)
PW = 512              # PSUM gemm tile width (j-cols)
FP32 = mybir.dt.float32
F16 = mybir.dt.float16
A = mybir.AluOpType


def build_program():
    nc = bacc.Bacc("TRN2", target_bir_lowering=False, debug=False,
                   enable_asserts=False, num_devices=1)

    xe_d = nc.dram_tensor("xe", [B_LOC, J, F], F16, kind="ExternalInput").ap()
    xo_d = nc.dram_tensor("xo", [B_LOC, JOP, F], F16, kind="ExternalInput").ap()
    br_d = nc.dram_tensor("btr", [F, H], F16, kind="ExternalInput").ap()
    bi_d = nc.dram_tensor("bti", [F, H], F16, kind="ExternalInput").ap()
    br2_d = nc.dram_tensor("btr2", [F, H], F16, kind="ExternalInput").ap()
    bi2_d = nc.dram_tensor("bti2", [F, H], F16, kind="ExternalInput").ap()
    r2_d = nc.dram_tensor("r2vec", [H], FP32, kind="ExternalInput").ap()
    c_d = nc.dram_tensor("ctab", [H, J], F16, kind="ExternalInput").ap()
    s_d = nc.dram_tensor("stab", [H, J], F16, kind="ExternalInput").ap()
    p1_d = nc.dram_tensor("p1tab", [H, J], F16, kind="ExternalInput").ap()
    p2_d = nc.dram_tensor("p2tab", [H, J], F16, kind="ExternalInput").ap()
    gr0_d = nc.dram_tensor("ginr", [H], FP32, kind="ExternalInput").ap()
    gi0_d = nc.dram_tensor("gini", [H], FP32, kind="ExternalInput").ap()
    out_d = nc.dram_tensor("out", [B_LOC, H, L], FP32, kind="ExternalOutput").ap()

    with tile.TileContext(nc) as tc, ExitStack() as ctx:
        singles = ctx.enter_context(tc.tile_pool(name="singles", bufs=1))
        xt_pool = ctx.enter_context(tc.tile_pool(name="xt", bufs=1))
        tab_pool = ctx.enter_context(tc.tile_pool(name="tabs", bufs=2))
        u_pool = ctx.enter_context(tc.tile_pool(name="u", bufs=2))
        work = ctx.enter_context(tc.tile_pool(name="work", bufs=2))
        ps_mm = ctx.enter_context(tc.tile_pool(name="ps_mm", bufs=2, space="PSUM"))

        # weights: [128 f-part, FG, H]
        wr = singles.tile([128, FG, H], F16)
        wi = singles.tile([128, FG, H], F16)
        wr2 = singles.tile([128, FG, H], F16)
        wi2 = singles.tile([128, FG, H], F16)
        nc.sync.dma_start(out=wr, in_=br_d.rearrange("(fg p) h -> p fg h", p=128))
        nc.sync.dma_start(out=wi, in_=bi_d.rearrange("(fg p) h -> p fg h", p=128))
        nc.sync.dma_start(out=wr2, in_=br2_d.rearrange("(fg p) h -> p fg h", p=128))
        nc.sync.dma_start(out=wi2, in_=bi2_d.rearrange("(fg p) h -> p fg h", p=128))

        r2_s = singles.tile([128, HG], FP32)
        gr0_s = singles.tile([128, HG], FP32)
        gi0_s = singles.tile([128, HG], FP32)
        nc.sync.dma_start(out=r2_s, in_=r2_d.rearrange("(hg p) -> p hg", p=128))
        nc.sync.dma_start(out=gr0_s, in_=gr0_d.rearrange("(hg p) -> p hg", p=128))
        nc.sync.dma_start(out=gi0_s, in_=gi0_d.rearrange("(hg p) -> p hg", p=128))
        ones = singles.tile([128, W], FP32)
        nc.vector.memset(ones, 1.0)
        r2bc = singles.tile([128, HG, W], FP32)
        for hg in range(HG):
            nc.vector.tensor_scalar(r2bc[:, hg, :], ones, r2_s[:, hg:hg + 1],
                                    None, op0=A.mult)

        # x streams transposed into SBUF: even [128, J], odd [128, JOP]
        xte = [[xt_pool.tile([128, J], F16, tag=f"xte{b}_{fg}",
                             name=f"xte{b}_{fg}")
                for fg in range(FG)] for b in range(B_LOC)]
        xto = [[xt_pool.tile([128, JOP], F16, tag=f"xto{b}_{fg}",
                             name=f"xto{b}_{fg}")
                for fg in range(FG)] for b in range(B_LOC)]
        for b in range(B_LOC):
            for fg in range(FG):
                fsl = slice(fg * 128, (fg + 1) * 128)
                nc.sync.dma_start_transpose(xte[b][fg], xe_d[b, :, fsl])
                nc.sync.dma_start_transpose(xto[b][fg], xo_d[b, :, fsl])

        for hg in range(HG):
            hsl = slice(hg * 128, (hg + 1) * 128)
            # tables for this hg, full J width
            ct = tab_pool.tile([128, J], F16, tag="ct")
            st = tab_pool.tile([128, J], F16, tag="st")
            p1t = tab_pool.tile([128, J], F16, tag="p1t")
            p2t = tab_pool.tile([128, J], F16, tag="p2t")
            nc.sync.dma_start(out=ct, in_=c_d[hsl, :])
            nc.sync.dma_start(out=st, in_=s_d[hsl, :])
            nc.sync.dma_start(out=p1t, in_=p1_d[hsl, :])
            nc.sync.dma_start(out=p2t, in_=p2_d[hsl, :])

            for b in range(B_LOC):
                gprev = None
                for jt in range(NW):
                    j0 = jt * W
                    ur_sb = u_pool.tile([128, W], F16, tag="ur_sb")
                    ui_sb = u_pool.tile([128, W], F16, tag="ui_sb")
                    uo_sb = u_pool.tile([128, W], F16, tag="uo_sb")
                    for ps in range(W // PW):
                        p0 = j0 + ps * PW
                        psl = slice(p0, p0 + PW)
                        psl1 = slice(p0 + 1, p0 + PW + 1)
                        ssl = slice(ps * PW, (ps + 1) * PW)
                        pa = ps_mm.tile([128, PW], FP32, tag="pa")
                        pb = ps_mm.tile([128, PW], FP32, tag="pb")
                        pc = ps_mm.tile([128, PW], FP32, tag="pc")
                        for fg in range(FG):
                            nc.tensor.matmul(pa, wr2[:, fg, hsl],
                                             xto[b][fg][:, psl],
                                             start=(fg == 0), stop=False)
                        for fg in range(FG):
                            nc.tensor.matmul(pa, wr[:, fg, hsl],
                                             xte[b][fg][:, psl],
                                             start=False, stop=(fg == FG - 1))
                        for fg in range(FG):
                            nc.tensor.matmul(pb, wi2[:, fg, hsl],
                                             xto[b][fg][:, psl],
                                             start=(fg == 0), stop=False)
                        for fg in range(FG):
                            nc.tensor.matmul(pb, wi[:, fg, hsl],
                                             xte[b][fg][:, psl],
                                             start=False, stop=(fg == FG - 1))
                        for fg in range(FG):
                            nc.tensor.matmul(pc, wr[:, fg, hsl],
                                             xto[b][fg][:, psl1],
                                             start=(fg == 0), stop=(fg == FG - 1))
                        nc.scalar.copy(out=ur_sb[:, ssl], in_=pa)
                        nc.scalar.copy(out=ui_sb[:, ssl], in_=pb)
                        nc.scalar.copy(out=uo_sb[:, ssl], in_=pc)

                    jsl = slice(j0, j0 + W)
                    cw = ct[:, jsl]
                    sw = st[:, jsl]
                    # input rotation: v = e^{-i*2theta(j+1)} u~
                    t1 = work.tile([128, W], F16, tag="t1")
                    t2 = work.tile([128, W], F16, tag="t2")
                    t3 = work.tile([128, W], F16, tag="t3")
                    t4 = work.tile([128, W], F16, tag="t4")
                    vr = work.tile([128, W], F16, tag="vr")
                    vi = work.tile([128, W], F16, tag="vi")
                    nc.vector.tensor_mul(t1, cw, ur_sb)
                    nc.gpsimd.tensor_tensor(t2, sw, ui_sb, op=A.mult)
                    nc.vector.tensor_add(vr, t1, t2)
                    nc.gpsimd.tensor_tensor(t3, cw, ui_sb, op=A.mult)
                    nc.gpsimd.tensor_tensor(t4, sw, ur_sb, op=A.mult)
                    nc.vector.tensor_sub(vi, t3, t4)

                    gr = work.tile([128, W], F16, tag="gr")
                    gi = work.tile([128, W], F16, tag="gi")
                    if jt == 0:
                        init_r = gr0_s[:, hg:hg + 1]
                        init_i = gi0_s[:, hg:hg + 1]
                    else:
                        gr_p, gi_p = gprev
                        init_r = gr_p[:, W - 1:W]
                        init_i = gi_p[:, W - 1:W]
                    nc.vector.tensor_tensor_scan(gr, r2bc[:, hg, :], vr, init_r,
                                                 op0=A.mult, op1=A.add)
                    nc.vector.tensor_tensor_scan(gi, r2bc[:, hg, :], vi, init_i,
                                                 op0=A.mult, op1=A.add)
                    gprev = (gr, gi)

                    # output: even t=2j -> C.Gr - S.Gi ; odd -> P1.Gr-P2.Gi+uo
                    res = work.tile([128, 2 * W], FP32, tag="res")
                    res3 = res.rearrange("p (j two) -> p j two", two=2)
                    o1 = work.tile([128, W], F16, tag="t1", name="o1")
                    o2 = work.tile([128, W], F16, tag="t2", name="o2")
                    o3 = work.tile([128, W], F16, tag="t3", name="o3")
                    o4 = work.tile([128, W], F16, tag="t4", name="o4")
                    o5 = work.tile([128, W], F16, tag="vr", name="o5")
                    nc.vector.tensor_mul(o1, cw, gr)
                    nc.gpsimd.tensor_tensor(o2, sw, gi, op=A.mult)
                    nc.vector.tensor_sub(res3[:, :, 0], o1, o2)
                    nc.vector.tensor_mul(o3, p1t[:, jsl], gr)
                    nc.gpsimd.tensor_tensor(o4, p2t[:, jsl], gi, op=A.mult)
                    nc.vector.tensor_sub(o5, o3, o4)
                    nc.vector.tensor_add(res3[:, :, 1], o5, uo_sb)

                    nc.sync.dma_start(
                        out=out_d[b, hsl, 2 * j0:2 * (j0 + W)], in_=res)

    nc.compile()
    return nc


def _prepare(inputs):
    x = np.asarray(inputs["x"], dtype=np.float32)
    B_real = np.asarray(inputs["B_real"], dtype=np.float64)
    B_img = np.asarray(inputs["B_img"], dtype=np.float64)
    nu = np.asarray(inputs["nu"], dtype=np.float64)
    theta = np.asarray(inputs["theta"], dtype=np.float64)
    delta = np.asarray(inputs["delta"], dtype=np.float64)
    h0r = np.asarray(inputs["h0_real"], dtype=np.float64)
    h0i = np.asarray(inputs["h0_img"], dtype=np.float64)

    r = np.exp(-np.exp(nu))
    btr = B_real * delta[None, :]
    bti = B_img * delta[None, :]
    rc = r * np.cos(theta)
    rs = r * np.sin(theta)
    btr2 = btr * rc[None, :] - bti * rs[None, :]
    bti2 = btr * rs[None, :] + bti * rc[None, :]

    jj = np.arange(J, dtype=np.float64)
    ang_c = theta[:, None] * (2.0 * jj + 2.0)[None, :]   # 2theta(j+1)
    ctab = np.cos(ang_c)
    stab = np.sin(ang_c)
    ang_p = theta[:, None] * (2.0 * jj + 3.0)[None, :]   # theta(2j+3)
    p1 = r[:, None] * np.cos(ang_p)
    p2 = r[:, None] * np.sin(ang_p)

    # G_{-1} = h0 / lam = h0 * e^{-i theta} / r
    ginr = (h0r * np.cos(theta) + h0i * np.sin(theta)) / r
    gini = (h0i * np.cos(theta) - h0r * np.sin(theta)) / r

    xh = x.astype(np.float16)
    xe = np.ascontiguousarray(xh[:, 0::2, :])                      # (B, J, F)
    xo = np.zeros((B, JOP, F), dtype=np.float16)
    xo[:, 1:J + 1, :] = xh[:, 1::2, :]                             # slot p = x_{2p-1}

    return dict(
        btr=btr.astype(np.float16), bti=bti.astype(np.float16),
        btr2=btr2.astype(np.float16), bti2=bti2.astype(np.float16),
        r2vec=(r * r).astype(np.float32),
        ctab=ctab.astype(np.float16), stab=stab.astype(np.float16),
        p1tab=p1.astype(np.float16), p2tab=p2.astype(np.float16),
        ginr=ginr.astype(np.float32), gini=gini.astype(np.float32),
    ), xe, xo


_NC_CACHE = {}


def get_program():
    if "nc" not in _NC_CACHE:
        _NC_CACHE["nc"] = build_program()
    return _NC_CACHE["nc"]


def make_in_maps(inputs):
    shared, xe, xo = _prepare(inputs)
    return [dict(xe=np.ascontiguousarray(xe[c * B_LOC:(c + 1) * B_LOC]),
                 xo=np.ascontiguousarray(xo[c * B_LOC:(c + 1) * B_LOC]),
                 **shared)
            for c in range(N_CORES)]


def kernel(**inputs) -> np.ndarray:
    from concourse.bass_utils import run_bass_kernel_spmd

    nc = get_program()
    in_maps = make_in_maps(inputs)
    res = run_bass_kernel_spmd(nc, in_maps, list(range(N_CORES)))
    out = np.empty((B, L, H), dtype=np.float32)
    for c in range(N_CORES):
        out[c * B_LOC:(c + 1) * B_LOC] = res.results[c]["out"].transpose(0, 2, 1)
    return out


# revision 8
# speedup vs baseline: 1.8143x; 1.1049x over previous
"""LRU (diagonal complex linear recurrence) Trainium2 Bass kernel, v4.

Math (per batch b, channel h, time t = 0..L-1):
    u_t   = delta * (x_t @ B_real + i * x_t @ B_img)
    h_t   = lam * h_{t-1} + u_t,   h_{-1} = h0,  lam = r e^{i theta}
    out_t = Re(h_t)

v4 structure — time-decimated (m=2) polar scan with the pair-combine
folded into the GEMM:
  E_j := h_{2j} obeys E_j = lam^2 E_{j-1} + u~_j with
  u~_j = lam*u_{2j-1} + u_{2j} = x_{2j-1} @ (lam*Bd) + x_{2j} @ Bd.
  So a GEMM over pairs (x_{2j-1}, x_{2j}) with weight sets
  {btr2,bti2} = lam*(btr+i*bti) and {btr,bti} yields u~ directly at
  half resolution. Polar trick on lam^2 = r^2 e^{i*2theta}:
  E_j = e^{i*2theta(j+1)} G_j,  G_j = r^2 G_{j-1} + e^{-i*2theta(j+1)} u~_j,
  G_{-1} = h0 / lam  (host-computed, fp32).
  Even outputs: out_{2j}  = C.Gr - S.Gi           (C,S = cos/sin 2theta(j+1))
  Odd  outputs: out_{2j+1} = P1.Gr - P2.Gi + Re(u_{2j+1})
                (P1,P2 = r*cos/sin(theta(2j+3)); Re(u_odd) from a third
                 partial GEMM x_{2j+1} @ btr).
  Scans run on DVE at half resolution (scan is the only serial resource);
  rotations split DVE/GpSimd fp16 2x; PSUM->SBUF staging on ScalarE.
  Output written as [b, h, t] (contiguous 8KB rows), transposed on host.

Sharding: batch-parallel over 8 cores (2 batch elements each), SPMD.
"""

from contextlib import ExitStack

import numpy as np

import concourse.bass as bass
import concourse.tile as tile
from concourse import bacc, mybir

B, L, F, H = 16, 4096, 512, 512
N_CORES = 8
B_LOC = B // N_CORES
HG = H // 128
FG = F // 128
J = L // 2            # half-res length
JO = J + 1            # odd x stream incl. left pad
JOP = 2064            # padded to multiple of 16 for xbar transpose
W = 1024              # elementwise tile width (j-cols)
NW = J // W           # 2 elementwise tiles per (hg, b)
PW = 512              # PSUM gemm tile width (j-cols)
FP32 = mybir.dt.float32
F16 = mybir.dt.float16
A = mybir.AluOpType


def build_program():
    nc = bacc.Bacc("TRN2", target_bir_lowering=False, debug=False,
                   enable_asserts=False, num_devices=1)

    xe_d = nc.dram_tensor("xe", [B_LOC, J, F], F16, kind="ExternalInput").ap()
    xo_d = nc.dram_tensor("xo", [B_LOC, JOP, F], F16, kind="ExternalInput").ap()
    br_d = nc.dram_tensor("btr", [F, H], F16, kind="ExternalInput").ap()
    bi_d = nc.dram_tensor("bti", [F, H], F16, kind="ExternalInput").ap()
    br2_d = nc.dram_tensor("btr2", [F, H], F16, kind="ExternalInput").ap()
    bi2_d = nc.dram_tensor("bti2", [F, H], F16, kind="ExternalInput").ap()
    r2_d = nc.dram_tensor("r2vec", [H], FP32, kind="ExternalInput").ap()
    c_d = nc.dram_tensor("ctab", [H, J], F16, kind="ExternalInput").ap()
    s_d = nc.dram_tensor("stab", [H, J], F16, kind="ExternalInput").ap()
    p1_d = nc.dram_tensor("p1tab", [H, J], F16, kind="ExternalInput").ap()
    p2_d = nc.dram_tensor("p2tab", [H, J], F16, kind="ExternalInput").ap()
    gr0_d = nc.dram_tensor("ginr", [H], FP32, kind="ExternalInput").ap()
    gi0_d = nc.dram_tensor("gini", [H], FP32, kind="ExternalInput").ap()
    oute_d = nc.dram_tensor("oute", [B_LOC, H, J], F16, kind="ExternalOutput").ap()
    outo_d = nc.dram_tensor("outo", [B_LOC, H, J], F16, kind="ExternalOutput").ap()

    with tile.TileContext(nc) as tc, ExitStack() as ctx:
        singles = ctx.enter_context(tc.tile_pool(name="singles", bufs=1))
        xt_pool = ctx.enter_context(tc.tile_pool(name="xt", bufs=1))
        tab_pool = ctx.enter_context(tc.tile_pool(name="tabs", bufs=2))
        u_pool = ctx.enter_context(tc.tile_pool(name="u", bufs=2))
        work = ctx.enter_context(tc.tile_pool(name="work", bufs=2))
        ps_mm = ctx.enter_context(tc.tile_pool(name="ps_mm", bufs=2, space="PSUM"))

        # weights: [128 f-part, FG, H]
        wr = singles.tile([128, FG, H], F16)
        wi = singles.tile([128, FG, H], F16)
        wr2 = singles.tile([128, FG, H], F16)
        wi2 = singles.tile([128, FG, H], F16)
        nc.sync.dma_start(out=wr, in_=br_d.rearrange("(fg p) h -> p fg h", p=128))
        nc.sync.dma_start(out=wi, in_=bi_d.rearrange("(fg p) h -> p fg h", p=128))
        nc.sync.dma_start(out=wr2, in_=br2_d.rearrange("(fg p) h -> p fg h", p=128))
        nc.sync.dma_start(out=wi2, in_=bi2_d.rearrange("(fg p) h -> p fg h", p=128))

        r2_s = singles.tile([128, HG], FP32)
        gr0_s = singles.tile([128, HG], FP32)
        gi0_s = singles.tile([128, HG], FP32)
        nc.sync.dma_start(out=r2_s, in_=r2_d.rearrange("(hg p) -> p hg", p=128))
        nc.sync.dma_start(out=gr0_s, in_=gr0_d.rearrange("(hg p) -> p hg", p=128))
        nc.sync.dma_start(out=gi0_s, in_=gi0_d.rearrange("(hg p) -> p hg", p=128))
        ones = singles.tile([128, W], FP32)
        nc.vector.memset(ones, 1.0)
        r2bc = singles.tile([128, HG, W], FP32)
        for hg in range(HG):
            nc.vector.tensor_scalar(r2bc[:, hg, :], ones, r2_s[:, hg:hg + 1],
                                    None, op0=A.mult)

        # x streams transposed into SBUF: even [128, J], odd [128, JOP]
        xte = [[xt_pool.tile([128, J], F16, tag=f"xte{b}_{fg}",
                             name=f"xte{b}_{fg}")
                for fg in range(FG)] for b in range(B_LOC)]
        xto = [[xt_pool.tile([128, JOP], F16, tag=f"xto{b}_{fg}",
                             name=f"xto{b}_{fg}")
                for fg in range(FG)] for b in range(B_LOC)]
        for b in range(B_LOC):
            for fg in range(FG):
                fsl = slice(fg * 128, (fg + 1) * 128)
                nc.sync.dma_start_transpose(xte[b][fg], xe_d[b, :, fsl])
                nc.sync.dma_start_transpose(xto[b][fg], xo_d[b, :, fsl])

        for hg in range(HG):
            hsl = slice(hg * 128, (hg + 1) * 128)
            # tables for this hg, full J width
            ct = tab_pool.tile([128, J], F16, tag="ct")
            st = tab_pool.tile([128, J], F16, tag="st")
            p1t = tab_pool.tile([128, J], F16, tag="p1t")
            p2t = tab_pool.tile([128, J], F16, tag="p2t")
            nc.sync.dma_start(out=ct, in_=c_d[hsl, :])
            nc.sync.dma_start(out=st, in_=s_d[hsl, :])
            nc.sync.dma_start(out=p1t, in_=p1_d[hsl, :])
            nc.sync.dma_start(out=p2t, in_=p2_d[hsl, :])

            for b in range(B_LOC):
                gprev = None
                for jt in range(NW):
                    j0 = jt * W
                    ur_sb = u_pool.tile([128, W], F16, tag="ur_sb")
                    ui_sb = u_pool.tile([128, W], F16, tag="ui_sb")
                    uo_sb = u_pool.tile([128, W], F16, tag="uo_sb")
                    for ps in range(W // PW):
                        p0 = j0 + ps * PW
                        psl = slice(p0, p0 + PW)
                        psl1 = slice(p0 + 1, p0 + PW + 1)
                        ssl = slice(ps * PW, (ps + 1) * PW)
                        pa = ps_mm.tile([128, PW], FP32, tag="pa")
                        pb = ps_mm.tile([128, PW], FP32, tag="pb")
                        pc = ps_mm.tile([128, PW], FP32, tag="pc")
                        for fg in range(FG):
                            nc.tensor.matmul(pa, wr2[:, fg, hsl],
                                             xto[b][fg][:, psl],
                                             start=(fg == 0), stop=False)
                        for fg in range(FG):
                            nc.tensor.matmul(pa, wr[:, fg, hsl],
                                             xte[b][fg][:, psl],
                                             start=False, stop=(fg == FG - 1))
                        for fg in range(FG):
                            nc.tensor.matmul(pb, wi2[:, fg, hsl],
                                             xto[b][fg][:, psl],
                                             start=(fg == 0), stop=False)
                        for fg in range(FG):
                            nc.tensor.matmul(pb, wi[:, fg, hsl],
                                             xte[b][fg][:, psl],
                                             start=False, stop=(fg == FG - 1))
                        for fg in range(FG):
                            nc.tensor.matmul(pc, wr[:, fg, hsl],
                                             xto[b][fg][:, psl1],
                                             start=(fg == 0), stop=(fg == FG - 1))
                        nc.scalar.copy(out=ur_sb[:, ssl], in_=pa)
                        nc.scalar.copy(out=ui_sb[:, ssl], in_=pb)
                        nc.scalar.copy(out=uo_sb[:, ssl], in_=pc)

                    jsl = slice(j0, j0 + W)
                    cw = ct[:, jsl]
                    sw = st[:, jsl]
                    # input rotation: v = e^{-i*2theta(j+1)} u~
                    t1 = work.tile([128, W], F16, tag="t1")
                    t2 = work.tile([128, W], F16, tag="t2")
                    t3 = work.tile([128, W], F16, tag="t3")
                    t4 = work.tile([128, W], F16, tag="t4")
                    vr = work.tile([128, W], F16, tag="vr")
                    vi = work.tile([128, W], F16, tag="vi")
                    nc.vector.tensor_mul(t1, cw, ur_sb)
                    nc.gpsimd.tensor_tensor(t2, sw, ui_sb, op=A.mult)
                    nc.vector.tensor_add(vr, t1, t2)
                    nc.vector.tensor_mul(t3, cw, ui_sb)
                    nc.gpsimd.tensor_tensor(t4, sw, ur_sb, op=A.mult)
                    nc.vector.tensor_sub(vi, t3, t4)

                    gr = work.tile([128, W], F16, tag="gr")
                    gi = work.tile([128, W], F16, tag="gi")
                    if jt == 0:
                        init_r = gr0_s[:, hg:hg + 1]
                        init_i = gi0_s[:, hg:hg + 1]
                    else:
                        gr_p, gi_p = gprev
                        init_r = gr_p[:, W - 1:W]
                        init_i = gi_p[:, W - 1:W]
                    nc.vector.tensor_tensor_scan(gr, r2bc[:, hg, :], vr, init_r,
                                                 op0=A.mult, op1=A.add)
                    nc.vector.tensor_tensor_scan(gi, r2bc[:, hg, :], vi, init_i,
                                                 op0=A.mult, op1=A.add)
                    gprev = (gr, gi)

                    # output: even t=2j -> C.Gr - S.Gi ; odd -> P1.Gr-P2.Gi+uo
                    o1 = work.tile([128, W], F16, tag="t1", name="o1")
                    o2 = work.tile([128, W], F16, tag="t2", name="o2")
                    o3 = work.tile([128, W], F16, tag="t3", name="o3")
                    o4 = work.tile([128, W], F16, tag="t4", name="o4")
                    o5 = work.tile([128, W], F16, tag="vr", name="o5")
                    res_e = work.tile([128, W], F16, tag="res_e")
                    res_o = work.tile([128, W], F16, tag="res_o")
                    nc.vector.tensor_mul(o1, cw, gr)
                    nc.gpsimd.tensor_tensor(o2, sw, gi, op=A.mult)
                    nc.vector.tensor_sub(res_e, o1, o2)
                    nc.vector.tensor_mul(o3, p1t[:, jsl], gr)
                    nc.vector.tensor_mul(o4, p2t[:, jsl], gi)
                    nc.vector.tensor_sub(o5, o3, o4)
                    nc.gpsimd.tensor_tensor(res_o, o5, uo_sb, op=A.add)

                    nc.sync.dma_start(out=oute_d[b, hsl, jsl], in_=res_e)
                    nc.sync.dma_start(out=outo_d[b, hsl, jsl], in_=res_o)

    nc.compile()
    return nc


def _prepare(inputs):
    x = np.asarray(inputs["x"], dtype=np.float32)
    B_real = np.asarray(inputs["B_real"], dtype=np.float64)
    B_img = np.asarray(inputs["B_img"], dtype=np.float64)
    nu = np.asarray(inputs["nu"], dtype=np.float64)
    theta = np.asarray(inputs["theta"], dtype=np.float64)
    delta = np.asarray(inputs["delta"], dtype=np.float64)
    h0r = np.asarray(inputs["h0_real"], dtype=np.float64)
    h0i = np.asarray(inputs["h0_img"], dtype=np.float64)

    r = np.exp(-np.exp(nu))
    btr = B_real * delta[None, :]
    bti = B_img * delta[None, :]
    rc = r * np.cos(theta)
    rs = r * np.sin(theta)
    btr2 = btr * rc[None, :] - bti * rs[None, :]
    bti2 = btr * rs[None, :] + bti * rc[None, :]

    jj = np.arange(J, dtype=np.float64)
    ang_c = theta[:, None] * (2.0 * jj + 2.0)[None, :]   # 2theta(j+1)
    ctab = np.cos(ang_c)
    stab = np.sin(ang_c)
    ang_p = theta[:, None] * (2.0 * jj + 3.0)[None, :]   # theta(2j+3)
    p1 = r[:, None] * np.cos(ang_p)
    p2 = r[:, None] * np.sin(ang_p)

    # G_{-1} = h0 / lam = h0 * e^{-i theta} / r
    ginr = (h0r * np.cos(theta) + h0i * np.sin(theta)) / r
    gini = (h0i * np.cos(theta) - h0r * np.sin(theta)) / r

    xh = x.astype(np.float16)
    xe = np.ascontiguousarray(xh[:, 0::2, :])                      # (B, J, F)
    xo = np.zeros((B, JOP, F), dtype=np.float16)
    xo[:, 1:J + 1, :] = xh[:, 1::2, :]                             # slot p = x_{2p-1}

    return dict(
        btr=btr.astype(np.float16), bti=bti.astype(np.float16),
        btr2=btr2.astype(np.float16), bti2=bti2.astype(np.float16),
        r2vec=(r * r).astype(np.float32),
        ctab=ctab.astype(np.float16), stab=stab.astype(np.float16),
        p1tab=p1.astype(np.float16), p2tab=p2.astype(np.float16),
        ginr=ginr.astype(np.float32), gini=gini.astype(np.float32),
    ), xe, xo


_NC_CACHE = {}


def get_program():
    if "nc" not in _NC_CACHE:
        _NC_CACHE["nc"] = build_program()
    return _NC_CACHE["nc"]


def make_in_maps(inputs):
    shared, xe, xo = _prepare(inputs)
    return [dict(xe=np.ascontiguousarray(xe[c * B_LOC:(c + 1) * B_LOC]),
                 xo=np.ascontiguousarray(xo[c * B_LOC:(c + 1) * B_LOC]),
                 **shared)
            for c in range(N_CORES)]


def kernel(**inputs) -> np.ndarray:
    from concourse.bass_utils import run_bass_kernel_spmd

    nc = get_program()
    in_maps = make_in_maps(inputs)
    res = run_bass_kernel_spmd(nc, in_maps, list(range(N_CORES)))
    out = np.empty((B, L, H), dtype=np.float32)
    for c in range(N_CORES):
        sl = slice(c * B_LOC, (c + 1) * B_LOC)
        out[sl, 0::2, :] = res.results[c]["oute"].transpose(0, 2, 1)
        out[sl, 1::2, :] = res.results[c]["outo"].transpose(0, 2, 1)
    return out


# revision 9
# speedup vs baseline: 2.3521x; 1.2965x over previous
"""LRU (diagonal complex linear recurrence) Trainium2 Bass kernel, v4.

Math (per batch b, channel h, time t = 0..L-1):
    u_t   = delta * (x_t @ B_real + i * x_t @ B_img)
    h_t   = lam * h_{t-1} + u_t,   h_{-1} = h0,  lam = r e^{i theta}
    out_t = Re(h_t)

v4 structure — time-decimated (m=2) polar scan with the pair-combine
folded into the GEMM:
  E_j := h_{2j} obeys E_j = lam^2 E_{j-1} + u~_j with
  u~_j = lam*u_{2j-1} + u_{2j} = x_{2j-1} @ (lam*Bd) + x_{2j} @ Bd.
  So a GEMM over pairs (x_{2j-1}, x_{2j}) with weight sets
  {btr2,bti2} = lam*(btr+i*bti) and {btr,bti} yields u~ directly at
  half resolution. Polar trick on lam^2 = r^2 e^{i*2theta}:
  E_j = e^{i*2theta(j+1)} G_j,  G_j = r^2 G_{j-1} + e^{-i*2theta(j+1)} u~_j,
  G_{-1} = h0 / lam  (host-computed, fp32).
  Even outputs: out_{2j}  = C.Gr - S.Gi           (C,S = cos/sin 2theta(j+1))
  Odd  outputs: out_{2j+1} = P1.Gr - P2.Gi + Re(u_{2j+1})
                (P1,P2 = r*cos/sin(theta(2j+3)); Re(u_odd) from a third
                 partial GEMM x_{2j+1} @ btr).
  Scans run on DVE at half resolution (scan is the only serial resource);
  rotations split DVE/GpSimd fp16 2x; PSUM->SBUF staging on ScalarE.
  Output written as [b, h, t] (contiguous 8KB rows), transposed on host.

Sharding: batch-parallel over 8 cores (2 batch elements each), SPMD.
"""

from contextlib import ExitStack

import numpy as np

import concourse.bass as bass
import concourse.tile as tile
from concourse import bacc, mybir

B, L, F, H = 16, 4096, 512, 512
N_CORES = 8
B_LOC = B // N_CORES
HG = H // 128
FG = F // 128
J = L // 2            # half-res length
JO = J + 1            # odd x stream incl. left pad
JOP = 2064            # padded to multiple of 16 for xbar transpose
W = 1024              # elementwise tile width (j-cols)
NW = J // W           # 2 elementwise tiles per (hg, b)
PW = 512              # PSUM gemm tile width (j-cols)
FP32 = mybir.dt.float32
F16 = mybir.dt.float16
A = mybir.AluOpType


def build_program():
    nc = bacc.Bacc("TRN2", target_bir_lowering=False, debug=False,
                   enable_asserts=False, num_devices=1)

    xe_d = nc.dram_tensor("xe", [B_LOC, J, F], F16, kind="ExternalInput").ap()
    xo_d = nc.dram_tensor("xo", [B_LOC, JOP, F], F16, kind="ExternalInput").ap()
    br_d = nc.dram_tensor("btr", [F, H], F16, kind="ExternalInput").ap()
    bi_d = nc.dram_tensor("bti", [F, H], F16, kind="ExternalInput").ap()
    br2_d = nc.dram_tensor("btr2", [F, H], F16, kind="ExternalInput").ap()
    bi2_d = nc.dram_tensor("bti2", [F, H], F16, kind="ExternalInput").ap()
    r2_d = nc.dram_tensor("r2vec", [H], FP32, kind="ExternalInput").ap()
    c_d = nc.dram_tensor("ctab", [H, J], F16, kind="ExternalInput").ap()
    s_d = nc.dram_tensor("stab", [H, J], F16, kind="ExternalInput").ap()
    p1_d = nc.dram_tensor("p1tab", [H, J], F16, kind="ExternalInput").ap()
    p2_d = nc.dram_tensor("p2tab", [H, J], F16, kind="ExternalInput").ap()
    gr0_d = nc.dram_tensor("ginr", [H], FP32, kind="ExternalInput").ap()
    gi0_d = nc.dram_tensor("gini", [H], FP32, kind="ExternalInput").ap()
    oute_d = nc.dram_tensor("oute", [B_LOC, H, J], F16, kind="ExternalOutput").ap()
    outo_d = nc.dram_tensor("outo", [B_LOC, H, J], F16, kind="ExternalOutput").ap()

    with tile.TileContext(nc) as tc, ExitStack() as ctx:
        singles = ctx.enter_context(tc.tile_pool(name="singles", bufs=1))
        xt_pool = ctx.enter_context(tc.tile_pool(name="xt", bufs=1))
        tab_pool = ctx.enter_context(tc.tile_pool(name="tabs", bufs=2))
        u_pool = ctx.enter_context(tc.tile_pool(name="u", bufs=2))
        work = ctx.enter_context(tc.tile_pool(name="work", bufs=2))
        ps_mm = ctx.enter_context(tc.tile_pool(name="ps_mm", bufs=2, space="PSUM"))

        # weights: [128 f-part, FG, H]
        wr = singles.tile([128, FG, H], F16)
        wi = singles.tile([128, FG, H], F16)
        wr2 = singles.tile([128, FG, H], F16)
        wi2 = singles.tile([128, FG, H], F16)
        nc.sync.dma_start(out=wr, in_=br_d.rearrange("(fg p) h -> p fg h", p=128))
        nc.sync.dma_start(out=wi, in_=bi_d.rearrange("(fg p) h -> p fg h", p=128))
        nc.sync.dma_start(out=wr2, in_=br2_d.rearrange("(fg p) h -> p fg h", p=128))
        nc.sync.dma_start(out=wi2, in_=bi2_d.rearrange("(fg p) h -> p fg h", p=128))

        r2_s = singles.tile([128, HG], FP32)
        gr0_s = singles.tile([128, HG], FP32)
        gi0_s = singles.tile([128, HG], FP32)
        nc.sync.dma_start(out=r2_s, in_=r2_d.rearrange("(hg p) -> p hg", p=128))
        nc.sync.dma_start(out=gr0_s, in_=gr0_d.rearrange("(hg p) -> p hg", p=128))
        nc.sync.dma_start(out=gi0_s, in_=gi0_d.rearrange("(hg p) -> p hg", p=128))
        ones = singles.tile([128, W], FP32)
        nc.vector.memset(ones, 1.0)
        r2bc = singles.tile([128, HG, W], FP32)
        for hg in range(HG):
            nc.vector.tensor_scalar(r2bc[:, hg, :], ones, r2_s[:, hg:hg + 1],
                                    None, op0=A.mult)

        # x streams transposed into SBUF: even [128, J], odd [128, JOP]
        xte = [[xt_pool.tile([128, J], F16, tag=f"xte{b}_{fg}",
                             name=f"xte{b}_{fg}")
                for fg in range(FG)] for b in range(B_LOC)]
        xto = [[xt_pool.tile([128, JOP], F16, tag=f"xto{b}_{fg}",
                             name=f"xto{b}_{fg}")
                for fg in range(FG)] for b in range(B_LOC)]
        for b in range(B_LOC):
            for fg in range(FG):
                fsl = slice(fg * 128, (fg + 1) * 128)
                nc.sync.dma_start_transpose(xte[b][fg], xe_d[b, :, fsl])
                nc.sync.dma_start_transpose(xto[b][fg], xo_d[b, :, fsl])

        for hg in range(HG):
            hsl = slice(hg * 128, (hg + 1) * 128)
            # tables for this hg, full J width
            ct = tab_pool.tile([128, J], F16, tag="ct")
            st = tab_pool.tile([128, J], F16, tag="st")
            p1t = tab_pool.tile([128, J], F16, tag="p1t")
            p2t = tab_pool.tile([128, J], F16, tag="p2t")
            nc.sync.dma_start(out=ct, in_=c_d[hsl, :])
            nc.sync.dma_start(out=st, in_=s_d[hsl, :])
            nc.sync.dma_start(out=p1t, in_=p1_d[hsl, :])
            nc.sync.dma_start(out=p2t, in_=p2_d[hsl, :])

            for b in range(B_LOC):
                gprev = None
                for jt in range(NW):
                    j0 = jt * W
                    ur_sb = u_pool.tile([128, W], F16, tag="ur_sb")
                    ui_sb = u_pool.tile([128, W], F16, tag="ui_sb")
                    uo_sb = u_pool.tile([128, W], F16, tag="uo_sb")
                    for ps in range(W // PW):
                        p0 = j0 + ps * PW
                        psl = slice(p0, p0 + PW)
                        psl1 = slice(p0 + 1, p0 + PW + 1)
                        ssl = slice(ps * PW, (ps + 1) * PW)
                        pa = ps_mm.tile([128, PW], FP32, tag="pa")
                        pb = ps_mm.tile([128, PW], FP32, tag="pb")
                        pc = ps_mm.tile([128, PW], FP32, tag="pc")
                        for fg in range(FG):
                            nc.tensor.matmul(pa, wr2[:, fg, hsl],
                                             xto[b][fg][:, psl],
                                             start=(fg == 0), stop=False)
                        for fg in range(FG):
                            nc.tensor.matmul(pa, wr[:, fg, hsl],
                                             xte[b][fg][:, psl],
                                             start=False, stop=(fg == FG - 1))
                        for fg in range(FG):
                            nc.tensor.matmul(pb, wi2[:, fg, hsl],
                                             xto[b][fg][:, psl],
                                             start=(fg == 0), stop=False)
                        for fg in range(FG):
                            nc.tensor.matmul(pb, wi[:, fg, hsl],
                                             xte[b][fg][:, psl],
                                             start=False, stop=(fg == FG - 1))
                        for fg in range(FG):
                            nc.tensor.matmul(pc, wr[:, fg, hsl],
                                             xto[b][fg][:, psl1],
                                             start=(fg == 0), stop=(fg == FG - 1))
                        nc.scalar.copy(out=ur_sb[:, ssl], in_=pa)
                        nc.scalar.copy(out=ui_sb[:, ssl], in_=pb)
                        nc.scalar.copy(out=uo_sb[:, ssl], in_=pc)

                    jsl = slice(j0, j0 + W)
                    cw = ct[:, jsl]
                    sw = st[:, jsl]
                    # input rotation: v = e^{-i*2theta(j+1)} u~
                    t1 = work.tile([128, W], F16, tag="t1")
                    t2 = work.tile([128, W], F16, tag="t2")
                    t3 = work.tile([128, W], F16, tag="t3")
                    t4 = work.tile([128, W], F16, tag="t4")
                    vr = work.tile([128, W], F16, tag="vr")
                    vi = work.tile([128, W], F16, tag="vi")
                    nc.vector.tensor_mul(t1, cw, ur_sb)
                    nc.vector.tensor_mul(t2, sw, ui_sb)
                    nc.vector.tensor_add(vr, t1, t2)
                    nc.vector.tensor_mul(t3, cw, ui_sb)
                    nc.vector.tensor_mul(t4, sw, ur_sb)
                    nc.vector.tensor_sub(vi, t3, t4)

                    gr = work.tile([128, W], F16, tag="gr")
                    gi = work.tile([128, W], F16, tag="gi")
                    if jt == 0:
                        init_r = gr0_s[:, hg:hg + 1]
                        init_i = gi0_s[:, hg:hg + 1]
                    else:
                        gr_p, gi_p = gprev
                        init_r = gr_p[:, W - 1:W]
                        init_i = gi_p[:, W - 1:W]
                    nc.vector.tensor_tensor_scan(gr, r2bc[:, hg, :], vr, init_r,
                                                 op0=A.mult, op1=A.add)
                    nc.vector.tensor_tensor_scan(gi, r2bc[:, hg, :], vi, init_i,
                                                 op0=A.mult, op1=A.add)
                    gprev = (gr, gi)

                    # output: even t=2j -> C.Gr - S.Gi ; odd -> P1.Gr-P2.Gi+uo
                    o1 = work.tile([128, W], F16, tag="t1", name="o1")
                    o2 = work.tile([128, W], F16, tag="t2", name="o2")
                    o3 = work.tile([128, W], F16, tag="t3", name="o3")
                    o4 = work.tile([128, W], F16, tag="t4", name="o4")
                    o5 = work.tile([128, W], F16, tag="vr", name="o5")
                    res_e = work.tile([128, W], F16, tag="res_e")
                    res_o = work.tile([128, W], F16, tag="res_o")
                    nc.vector.tensor_mul(o1, cw, gr)
                    nc.vector.tensor_mul(o2, sw, gi)
                    nc.vector.tensor_sub(res_e, o1, o2)
                    nc.vector.tensor_mul(o3, p1t[:, jsl], gr)
                    nc.vector.tensor_mul(o4, p2t[:, jsl], gi)
                    nc.vector.tensor_sub(o5, o3, o4)
                    nc.vector.tensor_add(res_o, o5, uo_sb)

                    nc.sync.dma_start(out=oute_d[b, hsl, jsl], in_=res_e)
                    nc.sync.dma_start(out=outo_d[b, hsl, jsl], in_=res_o)

    nc.compile()
    return nc


def _prepare(inputs):
    x = np.asarray(inputs["x"], dtype=np.float32)
    B_real = np.asarray(inputs["B_real"], dtype=np.float64)
    B_img = np.asarray(inputs["B_img"], dtype=np.float64)
    nu = np.asarray(inputs["nu"], dtype=np.float64)
    theta = np.asarray(inputs["theta"], dtype=np.float64)
    delta = np.asarray(inputs["delta"], dtype=np.float64)
    h0r = np.asarray(inputs["h0_real"], dtype=np.float64)
    h0i = np.asarray(inputs["h0_img"], dtype=np.float64)

    r = np.exp(-np.exp(nu))
    btr = B_real * delta[None, :]
    bti = B_img * delta[None, :]
    rc = r * np.cos(theta)
    rs = r * np.sin(theta)
    btr2 = btr * rc[None, :] - bti * rs[None, :]
    bti2 = btr * rs[None, :] + bti * rc[None, :]

    jj = np.arange(J, dtype=np.float64)
    ang_c = theta[:, None] * (2.0 * jj + 2.0)[None, :]   # 2theta(j+1)
    ctab = np.cos(ang_c)
    stab = np.sin(ang_c)
    ang_p = theta[:, None] * (2.0 * jj + 3.0)[None, :]   # theta(2j+3)
    p1 = r[:, None] * np.cos(ang_p)
    p2 = r[:, None] * np.sin(ang_p)

    # G_{-1} = h0 / lam = h0 * e^{-i theta} / r
    ginr = (h0r * np.cos(theta) + h0i * np.sin(theta)) / r
    gini = (h0i * np.cos(theta) - h0r * np.sin(theta)) / r

    xh = x.astype(np.float16)
    xe = np.ascontiguousarray(xh[:, 0::2, :])                      # (B, J, F)
    xo = np.zeros((B, JOP, F), dtype=np.float16)
    xo[:, 1:J + 1, :] = xh[:, 1::2, :]                             # slot p = x_{2p-1}

    return dict(
        btr=btr.astype(np.float16), bti=bti.astype(np.float16),
        btr2=btr2.astype(np.float16), bti2=bti2.astype(np.float16),
        r2vec=(r * r).astype(np.float32),
        ctab=ctab.astype(np.float16), stab=stab.astype(np.float16),
        p1tab=p1.astype(np.float16), p2tab=p2.astype(np.float16),
        ginr=ginr.astype(np.float32), gini=gini.astype(np.float32),
    ), xe, xo


_NC_CACHE = {}


def get_program():
    if "nc" not in _NC_CACHE:
        _NC_CACHE["nc"] = build_program()
    return _NC_CACHE["nc"]


def make_in_maps(inputs):
    shared, xe, xo = _prepare(inputs)
    return [dict(xe=np.ascontiguousarray(xe[c * B_LOC:(c + 1) * B_LOC]),
                 xo=np.ascontiguousarray(xo[c * B_LOC:(c + 1) * B_LOC]),
                 **shared)
            for c in range(N_CORES)]


def kernel(**inputs) -> np.ndarray:
    from concourse.bass_utils import run_bass_kernel_spmd

    nc = get_program()
    in_maps = make_in_maps(inputs)
    res = run_bass_kernel_spmd(nc, in_maps, list(range(N_CORES)))
    out = np.empty((B, L, H), dtype=np.float32)
    for c in range(N_CORES):
        sl = slice(c * B_LOC, (c + 1) * B_LOC)
        out[sl, 0::2, :] = res.results[c]["oute"].transpose(0, 2, 1)
        out[sl, 1::2, :] = res.results[c]["outo"].transpose(0, 2, 1)
    return out


# revision 11
# speedup vs baseline: 2.4953x; 1.0609x over previous
"""LRU (diagonal complex linear recurrence) Trainium2 Bass kernel, v4.

Math (per batch b, channel h, time t = 0..L-1):
    u_t   = delta * (x_t @ B_real + i * x_t @ B_img)
    h_t   = lam * h_{t-1} + u_t,   h_{-1} = h0,  lam = r e^{i theta}
    out_t = Re(h_t)

v4 structure — time-decimated (m=2) polar scan with the pair-combine
folded into the GEMM:
  E_j := h_{2j} obeys E_j = lam^2 E_{j-1} + u~_j with
  u~_j = lam*u_{2j-1} + u_{2j} = x_{2j-1} @ (lam*Bd) + x_{2j} @ Bd.
  So a GEMM over pairs (x_{2j-1}, x_{2j}) with weight sets
  {btr2,bti2} = lam*(btr+i*bti) and {btr,bti} yields u~ directly at
  half resolution. Polar trick on lam^2 = r^2 e^{i*2theta}:
  E_j = e^{i*2theta(j+1)} G_j,  G_j = r^2 G_{j-1} + e^{-i*2theta(j+1)} u~_j,
  G_{-1} = h0 / lam  (host-computed, fp32).
  Even outputs: out_{2j}  = C.Gr - S.Gi           (C,S = cos/sin 2theta(j+1))
  Odd  outputs: out_{2j+1} = P1.Gr - P2.Gi + Re(u_{2j+1})
                (P1,P2 = r*cos/sin(theta(2j+3)); Re(u_odd) from a third
                 partial GEMM x_{2j+1} @ btr).
  Scans run on DVE at half resolution (scan is the only serial resource);
  rotations split DVE/GpSimd fp16 2x; PSUM->SBUF staging on ScalarE.
  Output written as [b, h, t] (contiguous 8KB rows), transposed on host.

Sharding: batch-parallel over 8 cores (2 batch elements each), SPMD.
"""

from contextlib import ExitStack

import numpy as np

import concourse.bass as bass
import concourse.tile as tile
from concourse import bacc, mybir

B, L, F, H = 16, 4096, 512, 512
N_CORES = 8
B_LOC = B // N_CORES
HG = H // 128
FG = F // 128
J = L // 2            # half-res length
JO = J + 1            # odd x stream incl. left pad
JOP = 2064            # padded to multiple of 16 for xbar transpose
W = 1024              # elementwise tile width (j-cols)
NW = J // W           # 2 elementwise tiles per (hg, b)
PW = 512              # PSUM gemm tile width (j-cols)
FP32 = mybir.dt.float32
F16 = mybir.dt.float16
A = mybir.AluOpType


def build_program():
    nc = bacc.Bacc("TRN2", target_bir_lowering=False, debug=False,
                   enable_asserts=False, num_devices=1)

    xe_d = nc.dram_tensor("xe", [B_LOC, J, F], F16, kind="ExternalInput").ap()
    xo_d = nc.dram_tensor("xo", [B_LOC, JOP, F], F16, kind="ExternalInput").ap()
    br_d = nc.dram_tensor("btr", [F, H], F16, kind="ExternalInput").ap()
    bi_d = nc.dram_tensor("bti", [F, H], F16, kind="ExternalInput").ap()
    br2_d = nc.dram_tensor("btr2", [F, H], F16, kind="ExternalInput").ap()
    bi2_d = nc.dram_tensor("bti2", [F, H], F16, kind="ExternalInput").ap()
    r2_d = nc.dram_tensor("r2vec", [H], FP32, kind="ExternalInput").ap()
    c_d = nc.dram_tensor("ctab", [H, J], F16, kind="ExternalInput").ap()
    s_d = nc.dram_tensor("stab", [H, J], F16, kind="ExternalInput").ap()
    p1_d = nc.dram_tensor("p1tab", [H, J], F16, kind="ExternalInput").ap()
    p2_d = nc.dram_tensor("p2tab", [H, J], F16, kind="ExternalInput").ap()
    gr0_d = nc.dram_tensor("ginr", [H], FP32, kind="ExternalInput").ap()
    gi0_d = nc.dram_tensor("gini", [H], FP32, kind="ExternalInput").ap()
    oute_d = nc.dram_tensor("oute", [B_LOC, H, J], F16, kind="ExternalOutput").ap()
    outo_d = nc.dram_tensor("outo", [B_LOC, H, J], F16, kind="ExternalOutput").ap()

    with tile.TileContext(nc) as tc, ExitStack() as ctx:
        singles = ctx.enter_context(tc.tile_pool(name="singles", bufs=1))
        xt_pool = ctx.enter_context(tc.tile_pool(name="xt", bufs=1))
        tab_pool = ctx.enter_context(tc.tile_pool(name="tabs", bufs=2))
        u_pool = ctx.enter_context(tc.tile_pool(name="u", bufs=3))
        work = ctx.enter_context(tc.tile_pool(name="work", bufs=2))
        ps_mm = ctx.enter_context(tc.tile_pool(name="ps_mm", bufs=2, space="PSUM"))

        # weights: [128 f-part, FG, H]
        wr = singles.tile([128, FG, H], F16)
        wi = singles.tile([128, FG, H], F16)
        wr2 = singles.tile([128, FG, H], F16)
        wi2 = singles.tile([128, FG, H], F16)
        nc.sync.dma_start(out=wr, in_=br_d.rearrange("(fg p) h -> p fg h", p=128))
        nc.sync.dma_start(out=wi, in_=bi_d.rearrange("(fg p) h -> p fg h", p=128))
        nc.sync.dma_start(out=wr2, in_=br2_d.rearrange("(fg p) h -> p fg h", p=128))
        nc.sync.dma_start(out=wi2, in_=bi2_d.rearrange("(fg p) h -> p fg h", p=128))

        r2_s = singles.tile([128, HG], FP32)
        gr0_s = singles.tile([128, HG], FP32)
        gi0_s = singles.tile([128, HG], FP32)
        nc.sync.dma_start(out=r2_s, in_=r2_d.rearrange("(hg p) -> p hg", p=128))
        nc.sync.dma_start(out=gr0_s, in_=gr0_d.rearrange("(hg p) -> p hg", p=128))
        nc.sync.dma_start(out=gi0_s, in_=gi0_d.rearrange("(hg p) -> p hg", p=128))
        ones = singles.tile([128, W], FP32)
        nc.vector.memset(ones, 1.0)
        r2bc = singles.tile([128, HG, W], FP32)
        for hg in range(HG):
            nc.vector.tensor_scalar(r2bc[:, hg, :], ones, r2_s[:, hg:hg + 1],
                                    None, op0=A.mult)

        # hg=0 tables issued on sync BEFORE the (slow-issue) x transposes
        tabs0 = []
        for nm, src in (("ct", c_d), ("st", s_d), ("p1t", p1_d), ("p2t", p2_d)):
            t = tab_pool.tile([128, J], F16, tag=nm, name=f"{nm}0")
            nc.sync.dma_start(out=t, in_=src[0:128, :])
            tabs0.append(t)

        # x streams transposed into SBUF: even [128, J], odd [128, JOP]
        xte = [[xt_pool.tile([128, J], F16, tag=f"xte{b}_{fg}",
                             name=f"xte{b}_{fg}")
                for fg in range(FG)] for b in range(B_LOC)]
        xto = [[xt_pool.tile([128, JOP], F16, tag=f"xto{b}_{fg}",
                             name=f"xto{b}_{fg}")
                for fg in range(FG)] for b in range(B_LOC)]
        for b in range(B_LOC):
            for fg in range(FG):
                fsl = slice(fg * 128, (fg + 1) * 128)
                nc.sync.dma_start_transpose(xte[b][fg], xe_d[b, :, fsl])
                nc.sync.dma_start_transpose(xto[b][fg], xo_d[b, :, fsl])

        for hg in range(HG):
            hsl = slice(hg * 128, (hg + 1) * 128)
            if hg == 0:
                ct, st, p1t, p2t = tabs0
            else:
                ct = tab_pool.tile([128, J], F16, tag="ct")
                st = tab_pool.tile([128, J], F16, tag="st")
                p1t = tab_pool.tile([128, J], F16, tag="p1t")
                p2t = tab_pool.tile([128, J], F16, tag="p2t")
                nc.scalar.dma_start(out=ct, in_=c_d[hsl, :])
                nc.scalar.dma_start(out=st, in_=s_d[hsl, :])
                nc.scalar.dma_start(out=p1t, in_=p1_d[hsl, :])
                nc.scalar.dma_start(out=p2t, in_=p2_d[hsl, :])

            for b in range(B_LOC):
                gprev = None
                for jt in range(NW):
                    j0 = jt * W
                    ur_sb = u_pool.tile([128, W], F16, tag="ur_sb")
                    ui_sb = u_pool.tile([128, W], F16, tag="ui_sb")
                    uo_sb = u_pool.tile([128, W], F16, tag="uo_sb")
                    for ps in range(W // PW):
                        p0 = j0 + ps * PW
                        psl = slice(p0, p0 + PW)
                        psl1 = slice(p0 + 1, p0 + PW + 1)
                        ssl = slice(ps * PW, (ps + 1) * PW)
                        pa = ps_mm.tile([128, PW], FP32, tag="pa")
                        pb = ps_mm.tile([128, PW], FP32, tag="pb")
                        pc = ps_mm.tile([128, PW], FP32, tag="pc")
                        for fg in range(FG):
                            nc.tensor.matmul(pa, wr2[:, fg, hsl],
                                             xto[b][fg][:, psl],
                                             start=(fg == 0), stop=False)
                        for fg in range(FG):
                            nc.tensor.matmul(pa, wr[:, fg, hsl],
                                             xte[b][fg][:, psl],
                                             start=False, stop=(fg == FG - 1))
                        for fg in range(FG):
                            nc.tensor.matmul(pb, wi2[:, fg, hsl],
                                             xto[b][fg][:, psl],
                                             start=(fg == 0), stop=False)
                        for fg in range(FG):
                            nc.tensor.matmul(pb, wi[:, fg, hsl],
                                             xte[b][fg][:, psl],
                                             start=False, stop=(fg == FG - 1))
                        for fg in range(FG):
                            nc.tensor.matmul(pc, wr[:, fg, hsl],
                                             xto[b][fg][:, psl1],
                                             start=(fg == 0), stop=(fg == FG - 1))
                        nc.scalar.copy(out=ur_sb[:, ssl], in_=pa)
                        nc.scalar.copy(out=ui_sb[:, ssl], in_=pb)
                        nc.scalar.copy(out=uo_sb[:, ssl], in_=pc)

                    jsl = slice(j0, j0 + W)
                    cw = ct[:, jsl]
                    sw = st[:, jsl]
                    # input rotation: v = e^{-i*2theta(j+1)} u~
                    t1 = work.tile([128, W], F16, tag="t1")
                    t2 = work.tile([128, W], F16, tag="t2")
                    t3 = work.tile([128, W], F16, tag="t3")
                    t4 = work.tile([128, W], F16, tag="t4")
                    vr = work.tile([128, W], F16, tag="vr")
                    vi = work.tile([128, W], F16, tag="vi")
                    nc.vector.tensor_mul(t1, cw, ur_sb)
                    nc.vector.tensor_mul(t2, sw, ui_sb)
                    nc.vector.tensor_add(vr, t1, t2)
                    nc.vector.tensor_mul(t3, cw, ui_sb)
                    nc.vector.tensor_mul(t4, sw, ur_sb)
                    nc.vector.tensor_sub(vi, t3, t4)

                    gr = work.tile([128, W], F16, tag="gr")
                    gi = work.tile([128, W], F16, tag="gi")
                    if jt == 0:
                        init_r = gr0_s[:, hg:hg + 1]
                        init_i = gi0_s[:, hg:hg + 1]
                    else:
                        gr_p, gi_p = gprev
                        init_r = gr_p[:, W - 1:W]
                        init_i = gi_p[:, W - 1:W]
                    nc.vector.tensor_tensor_scan(gr, r2bc[:, hg, :], vr, init_r,
                                                 op0=A.mult, op1=A.add)
                    nc.vector.tensor_tensor_scan(gi, r2bc[:, hg, :], vi, init_i,
                                                 op0=A.mult, op1=A.add)
                    gprev = (gr, gi)

                    # output: even t=2j -> C.Gr - S.Gi ; odd -> P1.Gr-P2.Gi+uo
                    o1 = work.tile([128, W], F16, tag="t1", name="o1")
                    o2 = work.tile([128, W], F16, tag="t2", name="o2")
                    o3 = work.tile([128, W], F16, tag="t3", name="o3")
                    o4 = work.tile([128, W], F16, tag="t4", name="o4")
                    o5 = work.tile([128, W], F16, tag="vr", name="o5")
                    res_e = work.tile([128, W], F16, tag="res_e")
                    res_o = work.tile([128, W], F16, tag="res_o")
                    nc.vector.tensor_mul(o1, cw, gr)
                    nc.vector.tensor_mul(o2, sw, gi)
                    nc.vector.tensor_sub(res_e, o1, o2)
                    nc.vector.tensor_mul(o3, p1t[:, jsl], gr)
                    nc.vector.tensor_mul(o4, p2t[:, jsl], gi)
                    nc.vector.tensor_sub(o5, o3, o4)
                    nc.vector.tensor_add(res_o, o5, uo_sb)

                    nc.sync.dma_start(out=oute_d[b, hsl, jsl], in_=res_e)
                    nc.sync.dma_start(out=outo_d[b, hsl, jsl], in_=res_o)

    nc.compile()
    return nc


def _prepare(inputs):
    x = np.asarray(inputs["x"], dtype=np.float32)
    B_real = np.asarray(inputs["B_real"], dtype=np.float64)
    B_img = np.asarray(inputs["B_img"], dtype=np.float64)
    nu = np.asarray(inputs["nu"], dtype=np.float64)
    theta = np.asarray(inputs["theta"], dtype=np.float64)
    delta = np.asarray(inputs["delta"], dtype=np.float64)
    h0r = np.asarray(inputs["h0_real"], dtype=np.float64)
    h0i = np.asarray(inputs["h0_img"], dtype=np.float64)

    r = np.exp(-np.exp(nu))
    btr = B_real * delta[None, :]
    bti = B_img * delta[None, :]
    rc = r * np.cos(theta)
    rs = r * np.sin(theta)
    btr2 = btr * rc[None, :] - bti * rs[None, :]
    bti2 = btr * rs[None, :] + bti * rc[None, :]

    jj = np.arange(J, dtype=np.float64)
    ang_c = theta[:, None] * (2.0 * jj + 2.0)[None, :]   # 2theta(j+1)
    ctab = np.cos(ang_c)
    stab = np.sin(ang_c)
    ang_p = theta[:, None] * (2.0 * jj + 3.0)[None, :]   # theta(2j+3)
    p1 = r[:, None] * np.cos(ang_p)
    p2 = r[:, None] * np.sin(ang_p)

    # G_{-1} = h0 / lam = h0 * e^{-i theta} / r
    ginr = (h0r * np.cos(theta) + h0i * np.sin(theta)) / r
    gini = (h0i * np.cos(theta) - h0r * np.sin(theta)) / r

    xh = x.astype(np.float16)
    xe = np.ascontiguousarray(xh[:, 0::2, :])                      # (B, J, F)
    xo = np.zeros((B, JOP, F), dtype=np.float16)
    xo[:, 1:J + 1, :] = xh[:, 1::2, :]                             # slot p = x_{2p-1}

    return dict(
        btr=btr.astype(np.float16), bti=bti.astype(np.float16),
        btr2=btr2.astype(np.float16), bti2=bti2.astype(np.float16),
        r2vec=(r * r).astype(np.float32),
        ctab=ctab.astype(np.float16), stab=stab.astype(np.float16),
        p1tab=p1.astype(np.float16), p2tab=p2.astype(np.float16),
        ginr=ginr.astype(np.float32), gini=gini.astype(np.float32),
    ), xe, xo


_NC_CACHE = {}


def get_program():
    if "nc" not in _NC_CACHE:
        _NC_CACHE["nc"] = build_program()
    return _NC_CACHE["nc"]


def make_in_maps(inputs):
    shared, xe, xo = _prepare(inputs)
    return [dict(xe=np.ascontiguousarray(xe[c * B_LOC:(c + 1) * B_LOC]),
                 xo=np.ascontiguousarray(xo[c * B_LOC:(c + 1) * B_LOC]),
                 **shared)
            for c in range(N_CORES)]


def kernel(**inputs) -> np.ndarray:
    from concourse.bass_utils import run_bass_kernel_spmd

    nc = get_program()
    in_maps = make_in_maps(inputs)
    res = run_bass_kernel_spmd(nc, in_maps, list(range(N_CORES)))
    out = np.empty((B, L, H), dtype=np.float32)
    for c in range(N_CORES):
        sl = slice(c * B_LOC, (c + 1) * B_LOC)
        out[sl, 0::2, :] = res.results[c]["oute"].transpose(0, 2, 1)
        out[sl, 1::2, :] = res.results[c]["outo"].transpose(0, 2, 1)
    return out
